# revision 23
# baseline (speedup 1.0000x reference)
"""Trainium2 kernel for nn_NNK_Asym_Kernel (asymmetric random-feature module).

Math
----
The reference computes complex random features of ``xs = x / 256``:

    feat±[n, r] = exp(±a_r · p_nr − s_n · b_r − c_r) / sqrt(R)

with ``p = xs @ proj.T``, ``s_n = ||xs_n||²``, ``a = sqrt(1+4M)·xi``,
``b = xi²/2``, ``c_r = M·||g_r||²``, then contracts them against (exactly
computed) complex weight features.  Because ``x`` is scaled by ``1/DIM``,
every exponent on the x-side is tiny (|a·p| ≲ 2e-3, |s·b| ≲ 3e-4), so a
low-order expansion of the exponential is *far* below the float32 rounding
of the reference itself (measured: 1.9e-9 relative error for the linear
term vs the reference's own 1.2e-7 f32 evaluation noise).  The module
therefore collapses exactly (to ~1e-9) into an affine map

    out[n, o] = u0[o] + ssq_n · u1[o] + x[n, :] @ Meff[:, o]

with host-precomputed (float64) parameters:
    u0   — constant term (folds the exact complex weight features)
    u1   — coefficient of ssq_n = ||x_n||²   (contribution ~1e-4)
    Meff — (256, 1024) effective linear map; carries ALL of the
           per-element signal (std ~3e-4 on top of u0 ~ 1.669)

Device kernel (per core, 8192 rows, SPMD over 8 cores): the rank-256 GEMM
``x @ (Meff·S)`` — the only O(N·K·O) part — in bf16 with f32 PSUM
accumulation, emitted as float8-e4m3.  S is a power of 2 chosen from a
sampled max so the GEMM signal (~1e-6 here: the Wp−Wm weight structure
nearly cancels) sits in e4m3's normal range; fp8 transport then adds only
~2e-8 absolute error (~1e-8 relative to the full output, far below the
reference's own 1.2e-7 f32 rounding) while quartering the dominant output
DMA traffic.  The host un-scales, adds back the rank-1 affine part
(u0 + ssq·u1) in float32, and returns float32.

Perf notes (measured on trn2 via NTFF profiles):
  - bf16 everywhere on the PE: fp32 matmuls lower to a slow LOW_HIGH
    instruction pair (~2.3us per K=2 matmul vs ~220ns bf16).
  - All matmuls use full 128-partition contraction: partial-row-group
    matmuls (e.g. a K=4 chunk) do not register as "busy" in the PE
    activity monitor (HAM), which then keeps the PE clock throttled at
    1.2 GHz instead of 2.4 GHz for the whole kernel.
  - A 14-matmul dummy warm-up burst runs during the initial input DMAs so
    the HAM un-throttles before the real work starts.
  - Input loads are submitted from GpSimd's DMA queue, output stores from
    SyncE's: store submits block in-order on the copyback semaphore, and
    load submits queued behind them on the same engine starve the PE.
"""

import numpy as np
import ml_dtypes

import concourse.bass as bass
import concourse.tile as tile
from concourse import bacc, mybir
from concourse.bass_utils import run_bass_kernel_spmd

DIM = 256
R = 256
OUT_DIM = 1024
N = 65536
M = 0.001
N_CORES = 8
N_SHARD = N // N_CORES          # 8192 rows per core
BLK = 1024                      # rows handled per DMA block (8 subtiles)
SUB = 128                       # rows per matmul subtile
N_FREE = 512                    # matmul moving free dim (one PSUM bank)

_COMPILED = {}


def _build_nc():
    f32 = mybir.dt.float32
    f8e4 = mybir.dt.float8e4
    bf16 = mybir.dt.bfloat16

    nc = bacc.Bacc("TRN2", target_bir_lowering=False, debug=False,
                   num_devices=N_CORES)

    xT = nc.dram_tensor("xT", [DIM, N_SHARD], bf16, kind="ExternalInput")
    mrhs = nc.dram_tensor("mrhs", [DIM, OUT_DIM], bf16, kind="ExternalInput")
    out = nc.dram_tensor("out", [N_SHARD, OUT_DIM], f8e4, kind="ExternalOutput")

    n_blocks = N_SHARD // BLK
    subs_per_blk = BLK // SUB

    with tile.TileContext(nc) as tc:
        with (
            tc.tile_pool(name="const", bufs=1) as const,
            tc.tile_pool(name="xin", bufs=3) as xin,
            tc.tile_pool(name="psum", bufs=4, space="PSUM") as psum,
            tc.tile_pool(name="outp", bufs=10) as outp,
        ):
            m0 = const.tile([128, OUT_DIM], bf16, tag="m0")
            m1 = const.tile([128, OUT_DIM], bf16, tag="m1")
            dummy = const.tile([128, N_FREE], bf16, tag="dummy")

            # warm-up burst: the PE activity monitor needs ~4us of sustained
            # full-array work before it releases the 1.2GHz clock throttle.
            # Memset+matmuls are first in program order so they run while the
            # first input DMAs stream in.
            nc.gpsimd.memset(dummy[:], 0.0)
            wp = psum.tile([128, N_FREE], f32, tag="ps0", name="warmup_ps")
            for i in range(14):
                nc.tensor.matmul(wp[:], dummy[:, 0:128], dummy[:],
                                 start=(i == 0), stop=(i == 13))

            nc.gpsimd.dma_start(m0[:], mrhs[0:128, :])
            nc.gpsimd.dma_start(m1[:], mrhs[128:256, :])

            for blk in range(n_blocks):
                cs = blk * BLK
                xt0 = xin.tile([128, BLK], bf16, tag="xt0")
                xt1 = xin.tile([128, BLK], bf16, tag="xt1")
                nc.gpsimd.dma_start(xt0[:], xT[0:128, cs:cs + BLK])
                nc.gpsimd.dma_start(xt1[:], xT[128:256, cs:cs + BLK])

                for sub in range(subs_per_blk):
                    ns = sub * SUB
                    lhs0 = xt0[:, ns:ns + SUB]
                    lhs1 = xt1[:, ns:ns + SUB]
                    ps = [psum.tile([128, N_FREE], f32, tag=f"ps{h}",
                                    name=f"ps{h}_{blk}_{sub}")
                          for h in range(2)]
                    # chunk-outer / bank-inner: each stationary operand is
                    # loaded once and reused for both PSUM banks
                    for ci, lhs in enumerate((lhs0, lhs1)):
                        rhs = (m0, m1)[ci]
                        for h in range(2):
                            os_ = h * N_FREE
                            nc.tensor.matmul(ps[h][:], lhs,
                                             rhs[:, os_:os_ + N_FREE],
                                             start=(ci == 0), stop=(ci == 1))
                    ot = outp.tile([128, OUT_DIM], f8e4, tag="ot")
                    nc.vector.tensor_copy(ot[:, 0:N_FREE], ps[0][:])
                    nc.scalar.copy(ot[:, N_FREE:OUT_DIM], ps[1][:])
                    rs = cs + ns
                    nc.sync.dma_start(out[rs:rs + SUB, :], ot[:])

    nc.compile()
    return nc


def _precompute(x, input_weights, b, proj, xis_real, xis_imag):
    """Fold the entire module into (Meff, u0', u1', ssq_c) in float64."""
    f8 = np.float64
    xis = xis_real.astype(f8) + 1j * xis_imag.astype(f8)
    inv4 = 1.0 / DIM ** 0.25
    w = input_weights.astype(f8) * inv4
    bb = b.astype(f8) * inv4
    c = (1.0 / np.sqrt(2.0)) * (1.0 + 4.0 * M) ** (DIM / 4.0)
    projf = proj.astype(f8)
    g2 = (projf ** 2).sum(-1)
    scale = np.sqrt(1.0 + 4.0 * M)

    # exact complex weight features (the module's precomputed parameters)
    dotw = scale * (w @ projf.T) * xis[None, :]
    sqw = (w * w).sum(-1)
    corr = 0.5 * sqw[:, None] * (xis * xis)[None, :] + (M * g2)[None, :]
    ab_b = np.exp(xis[None, :] * bb[:, None])
    Wp = np.exp(dotw - corr) / np.sqrt(R) * ab_b
    Wm = np.exp(-dotw - corr) / np.sqrt(R) / ab_b

    E = np.exp(-M * g2) / np.sqrt(R)      # x-side per-feature constant
    c2 = c * c
    Sp = Wp + Wm
    Dp = Wp - Wm
    a = scale * xis
    bco = 0.5 * xis * xis

    u0 = c2 * (Sp.real * E[None, :]).sum(-1)                    # (O,)
    u1 = -c2 * ((Sp * bco[None, :]).real * E[None, :]).sum(-1)  # (O,) × s_n
    L = c2 * (Dp * a[None, :]).real * E[None, :]                # (O, R)
    Meff = (L @ projf).T / DIM                                  # (D, O)

    ssq = (x.astype(f8) ** 2).sum(-1)                           # (N,)
    u1p = u1 / DIM ** 2                                         # s_n = ssq/DIM²
    mssq = ssq.mean()
    u0p = u0 + mssq * u1p                                       # center ssq
    ssq_c = ssq - mssq
    return Meff, u0p, u1p, ssq_c


def _run(inputs, trace=False):
    x = np.asarray(inputs["x"], dtype=np.float32)
    input_weights = np.asarray(inputs["input_weights"], dtype=np.float32)
    b = np.asarray(inputs["b"], dtype=np.float32)
    proj = np.asarray(inputs["proj"], dtype=np.float32)
    xis_real = np.asarray(inputs["xis_real"], dtype=np.float32)
    xis_imag = np.asarray(inputs["xis_imag"], dtype=np.float32)

    Meff, u0p, u1p, ssq_c = _precompute(x, input_weights, b, proj,
                                        xis_real, xis_imag)

    # fp8-e4m3 output transport: scale the GEMM into e4m3's normal range
    # (power of 2 so the host decode is exact); sampled max + 64x headroom
    gs = x[:2048].astype(np.float32) @ Meff.astype(np.float32)
    gmax = float(np.abs(gs).max()) or 1.0
    out_scale = 2.0 ** np.floor(np.log2(64.0 / gmax))

    bf16 = ml_dtypes.bfloat16
    mrhs_np = (Meff * out_scale).astype(np.float32).astype(bf16)
    xb = x.astype(bf16)

    in_maps = []
    for k in range(N_CORES):
        rows = slice(k * N_SHARD, (k + 1) * N_SHARD)
        in_maps.append({
            "xT": np.ascontiguousarray(xb[rows].T),
            "mrhs": mrhs_np,
        })

    if "nc" not in _COMPILED:
        _COMPILED["nc"] = _build_nc()
    nc = _COMPILED["nc"]

    res = run_bass_kernel_spmd(nc, in_maps, core_ids=list(range(N_CORES)),
                               trace=trace)

    # host-side affine: out = gemm + u0 + ssq·u1 (rank-1, O(N·O) flops)
    u0f = u0p.astype(np.float32)
    u1f = u1p.astype(np.float32)
    out = np.empty((N, OUT_DIM), np.float32)
    for k in range(N_CORES):
        rows = slice(k * N_SHARD, (k + 1) * N_SHARD)
        shard = res.results[k]["out"].astype(np.float32)
        shard *= np.float32(1.0 / out_scale)
        shard += u0f[None, :]
        shard += ssq_c[rows].astype(np.float32)[:, None] * u1f[None, :]
        out[rows] = shard
    return out, res


def kernel(**inputs):
    out, _ = _run(inputs, trace=False)
    return out


# revision 26
# speedup vs baseline: 1.0044x; 1.0044x over previous
"""Trainium2 kernel for nn_NNK_Asym_Kernel (asymmetric random-feature module).

Math
----
The reference computes complex random features of ``xs = x / 256``:

    feat±[n, r] = exp(±a_r · p_nr − s_n · b_r − c_r) / sqrt(R)

with ``p = xs @ proj.T``, ``s_n = ||xs_n||²``, ``a = sqrt(1+4M)·xi``,
``b = xi²/2``, ``c_r = M·||g_r||²``, then contracts them against (exactly
computed) complex weight features.  Because ``x`` is scaled by ``1/DIM``,
every exponent on the x-side is tiny (|a·p| ≲ 2e-3, |s·b| ≲ 3e-4), so a
low-order expansion of the exponential is *far* below the float32 rounding
of the reference itself (measured: 1.9e-9 relative error for the linear
term vs the reference's own 1.2e-7 f32 evaluation noise).  The module
therefore collapses exactly (to ~1e-9) into an affine map

    out[n, o] = u0[o] + ssq_n · u1[o] + x[n, :] @ Meff[:, o]

with host-precomputed (float64) parameters:
    u0   — constant term (folds the exact complex weight features)
    u1   — coefficient of ssq_n = ||x_n||²   (contribution ~1e-4)
    Meff — (256, 1024) effective linear map; carries the per-element
           signal (~1e-6 scale, on top of u0 ~ 1.669)

Device kernel (per core, 8192 rows, SPMD over 8 cores): the rank-256 GEMM
``x @ (Meff·S)`` — the only O(N·K·O) part — in bf16 with f32 PSUM
accumulation, emitted as float8-e4m3.  S is a power of 2 chosen from a
sampled max so the GEMM signal (~1e-6 here: the Wp−Wm weight structure
nearly cancels) sits in e4m3's normal range; fp8 transport then adds only
~2e-8 absolute error (~1e-8 relative to the full output, far below the
reference's own 1.2e-7 f32 rounding) while quartering the dominant output
DMA traffic.  The host un-scales, adds back the rank-1 affine part
(u0 + ssq·u1) in float32, and returns float32.

Perf notes (measured on trn2 via NTFF profiles):
  - bf16 everywhere on the PE: fp32 matmuls lower to a slow LOW_HIGH
    instruction pair (~2.3us per K=2 matmul vs ~220ns bf16).
  - All matmuls use full 128-partition contraction: partial-row-group
    matmuls (e.g. a K=4 chunk) do not register as "busy" in the PE
    activity monitor (HAM), which then keeps the PE clock throttled at
    1.2 GHz instead of 2.4 GHz for the whole kernel.
  - A 14-matmul dummy warm-up burst runs during the initial input DMAs so
    the HAM un-throttles before the real work starts.
  - Input loads are submitted from GpSimd's DMA queue, output stores from
    SyncE's: store submits block in-order on the copyback semaphore, and
    load submits queued behind them on the same engine starve the PE.
"""

import numpy as np
import ml_dtypes

import concourse.bass as bass
import concourse.tile as tile
from concourse import bacc, mybir
from concourse.bass_utils import run_bass_kernel_spmd

DIM = 256
R = 256
OUT_DIM = 1024
N = 65536
M = 0.001
N_CORES = 8
N_SHARD = N // N_CORES          # 8192 rows per core
BLK = 1024                      # rows handled per DMA block (8 subtiles)
SUB = 128                       # rows per matmul subtile
N_FREE = 512                    # matmul moving free dim (one PSUM bank)

_COMPILED = {}


def _build_nc():
    f32 = mybir.dt.float32
    f8e4 = mybir.dt.float8e4
    bf16 = mybir.dt.bfloat16

    nc = bacc.Bacc("TRN2", target_bir_lowering=False, debug=False,
                   num_devices=N_CORES)

    xT = nc.dram_tensor("xT", [DIM, N_SHARD], bf16, kind="ExternalInput")
    mrhs = nc.dram_tensor("mrhs", [DIM, OUT_DIM], bf16, kind="ExternalInput")
    out = nc.dram_tensor("out", [N_SHARD, OUT_DIM], f8e4, kind="ExternalOutput")

    n_blocks = N_SHARD // BLK
    subs_per_blk = BLK // SUB

    with tile.TileContext(nc) as tc:
        with (
            tc.tile_pool(name="const", bufs=1) as const,
            tc.tile_pool(name="xin", bufs=3) as xin,
            tc.tile_pool(name="psum", bufs=4, space="PSUM") as psum,
            tc.tile_pool(name="outp", bufs=10) as outp,
        ):
            m0 = const.tile([128, OUT_DIM], bf16, tag="m0")
            m1 = const.tile([128, OUT_DIM], bf16, tag="m1")
            dummy = const.tile([128, N_FREE], bf16, tag="dummy")

            # warm-up burst: the PE activity monitor needs ~4us of sustained
            # full-array work before it releases the 1.2GHz clock throttle.
            # Memset+matmuls are first in program order so they run while the
            # first input DMAs stream in.
            nc.gpsimd.memset(dummy[:], 0.0)
            wp = psum.tile([128, N_FREE], f32, tag="ps0", name="warmup_ps")
            for i in range(14):
                nc.tensor.matmul(wp[:], dummy[:, 0:128], dummy[:],
                                 start=(i == 0), stop=(i == 13))

            nc.gpsimd.dma_start(m0[:], mrhs[0:128, :])
            nc.gpsimd.dma_start(m1[:], mrhs[128:256, :])

            for blk in range(n_blocks):
                cs = blk * BLK
                xt0 = xin.tile([128, BLK], bf16, tag="xt0")
                xt1 = xin.tile([128, BLK], bf16, tag="xt1")
                nc.gpsimd.dma_start(xt0[:], xT[0:128, cs:cs + BLK])
                nc.gpsimd.dma_start(xt1[:], xT[128:256, cs:cs + BLK])

                # Two subtiles (= one 256-row group) share one output tile
                # and one store DMA.  The host interleaves x rows per group
                # (evens then odds), so SBUF partition p of the pair tile
                # holds output rows gs+2p / gs+2p+1 — two consecutive DRAM
                # rows = 2KB-contiguous store descriptors (the fp8 rows
                # alone are only 1KB, which halves DMA efficiency).
                for pair in range(subs_per_blk // 2):
                    ot = outp.tile([128, 2 * OUT_DIM], f8e4, tag="ot")
                    for half in range(2):
                        ns = (2 * pair + half) * SUB
                        lhs0 = xt0[:, ns:ns + SUB]
                        lhs1 = xt1[:, ns:ns + SUB]
                        ps = [psum.tile([128, N_FREE], f32, tag=f"ps{h}",
                                        name=f"ps{h}_{blk}_{pair}_{half}")
                              for h in range(2)]
                        # chunk-outer / bank-inner: each stationary operand
                        # is loaded once and reused for both PSUM banks
                        for ci, lhs in enumerate((lhs0, lhs1)):
                            rhs = (m0, m1)[ci]
                            for h in range(2):
                                os_ = h * N_FREE
                                nc.tensor.matmul(ps[h][:], lhs,
                                                 rhs[:, os_:os_ + N_FREE],
                                                 start=(ci == 0), stop=(ci == 1))
                        ob = half * OUT_DIM
                        nc.vector.tensor_copy(ot[:, ob:ob + N_FREE], ps[0][:])
                        nc.scalar.copy(ot[:, ob + N_FREE:ob + OUT_DIM], ps[1][:])
                    gs = cs + pair * 2 * SUB
                    dst = out[gs:gs + 2 * SUB, :].rearrange(
                        "(p b) o -> p (b o)", b=2)
                    nc.sync.dma_start(dst, ot[:])

    nc.compile()
    return nc


def _precompute(x, input_weights, b, proj, xis_real, xis_imag):
    """Fold the entire module into (Meff, u0', u1', ssq_c) in float64."""
    f8 = np.float64
    xis = xis_real.astype(f8) + 1j * xis_imag.astype(f8)
    inv4 = 1.0 / DIM ** 0.25
    w = input_weights.astype(f8) * inv4
    bb = b.astype(f8) * inv4
    c = (1.0 / np.sqrt(2.0)) * (1.0 + 4.0 * M) ** (DIM / 4.0)
    projf = proj.astype(f8)
    g2 = (projf ** 2).sum(-1)
    scale = np.sqrt(1.0 + 4.0 * M)

    # exact complex weight features (the module's precomputed parameters)
    dotw = scale * (w @ projf.T) * xis[None, :]
    sqw = (w * w).sum(-1)
    corr = 0.5 * sqw[:, None] * (xis * xis)[None, :] + (M * g2)[None, :]
    ab_b = np.exp(xis[None, :] * bb[:, None])
    Wp = np.exp(dotw - corr) / np.sqrt(R) * ab_b
    Wm = np.exp(-dotw - corr) / np.sqrt(R) / ab_b

    E = np.exp(-M * g2) / np.sqrt(R)      # x-side per-feature constant
    c2 = c * c
    Sp = Wp + Wm
    Dp = Wp - Wm
    a = scale * xis
    bco = 0.5 * xis * xis

    u0 = c2 * (Sp.real * E[None, :]).sum(-1)                    # (O,)
    u1 = -c2 * ((Sp * bco[None, :]).real * E[None, :]).sum(-1)  # (O,) × s_n
    L = c2 * (Dp * a[None, :]).real * E[None, :]                # (O, R)
    Meff = (L @ projf).T / DIM                                  # (D, O)

    ssq = (x.astype(f8) ** 2).sum(-1)                           # (N,)
    u1p = u1 / DIM ** 2                                         # s_n = ssq/DIM²
    mssq = ssq.mean()
    u0p = u0 + mssq * u1p                                       # center ssq
    ssq_c = ssq - mssq
    return Meff, u0p, u1p, ssq_c


def _run(inputs, trace=False):
    x = np.asarray(inputs["x"], dtype=np.float32)
    input_weights = np.asarray(inputs["input_weights"], dtype=np.float32)
    b = np.asarray(inputs["b"], dtype=np.float32)
    proj = np.asarray(inputs["proj"], dtype=np.float32)
    xis_real = np.asarray(inputs["xis_real"], dtype=np.float32)
    xis_imag = np.asarray(inputs["xis_imag"], dtype=np.float32)

    Meff, u0p, u1p, ssq_c = _precompute(x, input_weights, b, proj,
                                        xis_real, xis_imag)

    # fp8-e4m3 output transport: scale the GEMM into e4m3's normal range
    # (power of 2 so the host decode is exact); sampled max + 64x headroom
    gs = x[:2048].astype(np.float32) @ Meff.astype(np.float32)
    gmax = float(np.abs(gs).max()) or 1.0
    out_scale = 2.0 ** np.floor(np.log2(64.0 / gmax))

    bf16 = ml_dtypes.bfloat16
    mrhs_np = (Meff * out_scale).astype(np.float32).astype(bf16)
    xb = x.astype(bf16)

    # per 256-row group, order rows evens-then-odds: SBUF partition p of a
    # subtile pair then maps to consecutive DRAM rows 2p/2p+1 on store
    perm = np.arange(N_SHARD).reshape(-1, SUB, 2).transpose(0, 2, 1).ravel()

    in_maps = []
    for k in range(N_CORES):
        rows = slice(k * N_SHARD, (k + 1) * N_SHARD)
        in_maps.append({
            "xT": np.ascontiguousarray(xb[rows][perm].T),
            "mrhs": mrhs_np,
        })

    if "nc" not in _COMPILED:
        _COMPILED["nc"] = _build_nc()
    nc = _COMPILED["nc"]

    res = run_bass_kernel_spmd(nc, in_maps, core_ids=list(range(N_CORES)),
                               trace=trace)

    # host-side affine: out = gemm + u0 + ssq·u1 (rank-1, O(N·O) flops)
    u0f = u0p.astype(np.float32)
    u1f = u1p.astype(np.float32)
    out = np.empty((N, OUT_DIM), np.float32)
    for k in range(N_CORES):
        rows = slice(k * N_SHARD, (k + 1) * N_SHARD)
        shard = res.results[k]["out"].astype(np.float32)
        shard *= np.float32(1.0 / out_scale)
        shard += u0f[None, :]
        shard += ssq_c[rows].astype(np.float32)[:, None] * u1f[None, :]
        out[rows] = shard
    return out, res


def kernel(**inputs):
    out, _ = _run(inputs, trace=False)
    return out
